# revision 45
# baseline (speedup 1.0000x reference)
"""BiGNN message-passing kernel for Trainium2 (8 NeuronCores, Bass/Tile).

Reference computation (N=100000 nodes, E=600000 edges, D=128):
    msgs = vals[:, None] * features[cols]            # gather + scale
    x    = segment_sum(msgs, rows)                   # scatter-add to rows
    out  = (features + x) @ W1 + b1 + (x * features) @ W2 + b2

Sharding: destination nodes (rows) are sharded across the 8 cores, 12500
each; `features` is replicated into every core's HBM, so the per-edge
source gather is core-local (no collectives).

The critical path is GPSIMD (SWDGE) descriptor generation for the
per-edge feature gather: ~2.3 ns/index, strictly serialized on the one
POOL engine (each InstDMAGatherAnt activates only the Q7 core pair of
its queue).  Everything else is arranged to hide underneath it:

  * gathered source features G: fp8e3m4 table laid out [cc+1, 2, 128]
    (payload in [:, 0, :], row cc all-zero for padding slots), gathered
    as 128B elements from 256B-stride rows via a raw InstDMAGatherAnt.
  * edge slots are packed DENSELY per (group, chunk) section, sorted by
    destination tile: blocks of 128 slots may straddle tile boundaries,
    cutting gather padding from 25% to ~8% (descgen is ~2.1ns/slot, so
    padding is pure critical-path loss).  The matmul schedule is the
    UNION over the 8 cores of (block, tile) pieces; a core lacking a
    piece gets an all-zero S block there (contributes nothing).
  * the one-hot scatter matrices S (S[slot, dst] = val, one 128x128 fp8
    block per piece) are built on the HOST and STREAMED from HBM on the
    SP HWDGE ring instead of being built on DVE (which used to be a
    second ~235us serial bottleneck contending with GPSIMD for the
    shared POOL SBUF port).
  * idx16 loads in per-group slices two groups ahead, so no gather ever
    waits on a bulk index transfer; the group schedule tapers to
    [4,3,2,1] tiles at the end so the post-descgen PE/epilogue tail is
    a few us instead of ~20.

Rejected variants (measured slower): building S on DVE (the original
281us baseline); 12/16-tile groups (S/G buffer pressure); trailing
negative-index trimming with per-core counts in num_idxs_reg — correct,
but the ucode's scalar trailing-scan costs more than the skipped pad
blocks save (231us), and with the count left at the padded value the
decode-side ring reservation mismatches the written descriptors and
wedges the device.

The segment-sum runs on TensorE, one matmul per piece:

    xT[f, d] += G[e, f].T @ S[e, d]        (fp8 x fp8 -> f32 psum)

Finished xT psums are evicted to SBUF in fp16 by the scalar engine, and
the dense epilogue for group g-1 is emitted inside group g:

    outT = W1.T @ (fT + xT) + W2.T @ (xT * fT) + (b1 + b2)

featT / outT move in fp16 on the ACT ring; the host transposes and
upcasts per-core outputs back to fp32.
"""

import numpy as np

P = 128
D = 128
N_NODES = 100000
N_EDGES = 600000
N_CORES = 8
NCHUNKS = 4  # feature-table column chunks (int16 index reach)
# dest tiles per gather/store group: small first groups fill the pipeline
# fast, small last groups shrink the post-descgen tail (the PE/epilogue
# work that serializes after the final gather)
GROUP_SIZES = [8] * 11 + [4, 3, 2, 1]

_LAST_RESULTS = None  # BassKernelResults of the most recent run (for test.py)


def _prep(rows, cols, vals, n_nodes, n_cores):
    """Host-side edge reorganization into the shared block schedule.

    Returns (sched, per_core):
      sched:
        tiles/npc/cc/ngroups/TOT/NP plus per-group gather sections and
        the shared matmul piece schedule (union over cores).
      per_core[c]:
        idx16 [128, TOT/16] int16   gather indices (pad -> zero row cc)
        S8    [128, NP*128] fp8e3m4 one-hot*val scatter blocks
    """
    import ml_dtypes

    npc = n_nodes // n_cores
    tiles = (npc + P - 1) // P
    assert sum(GROUP_SIZES) == tiles, (sum(GROUP_SIZES), tiles)
    ngroups = len(GROUP_SIZES)
    nsec = ngroups * NCHUNKS
    cc = n_nodes // NCHUNKS
    assert n_nodes % NCHUNKS == 0
    g_bounds = np.concatenate([[0], np.cumsum(GROUP_SIZES)])
    group_of_tile = np.repeat(np.arange(ngroups), GROUP_SIZES)

    rows = np.asarray(rows, dtype=np.int64)
    cols = np.asarray(cols, dtype=np.int64)
    vals = np.asarray(vals, dtype=np.float32)

    core = rows // npc
    local = rows - core * npc
    t_all = local // P
    dit_all = (local - t_all * P).astype(np.int64)
    j_all = (cols // cc).astype(np.int64)

    # int16 reach (32767 rows at 256B stride) exceeds the 25000-row chunk
    # by HALO rows: an edge with source in the first HALO rows of chunk j
    # can instead be gathered from chunk j-1's (halo-extended) table.
    # Greedily shift such edges one chunk down so each (group, chunk)
    # section packs tighter against the 128-slot block quantum across all
    # 8 cores -- every padded slot costs ~2.1ns of serialized GPSIMD
    # descriptor generation on every core.
    HALO = 32767 - cc
    flex = (j_all >= 1) & ((cols - j_all * cc) < HALO)
    grp_all = group_of_tile[t_all]
    cell = ((core * ngroups) + grp_all) * NCHUNKS + j_all  # (c, g, j)
    ncell = n_cores * ngroups * NCHUNKS
    a = np.bincount(cell, minlength=ncell).reshape(n_cores, ngroups, NCHUNKS)
    f = np.bincount(cell[flex], minlength=ncell).reshape(
        n_cores, ngroups, NCHUNKS
    )
    # move counts x[c, g, j]: edges shifted from (g, j) to (g, j-1).
    # No section may exceed 13 blocks (1664 slots): larger gathers
    # overflow the per-queue SWDGE descriptor ring and the Q7 stalls
    # mid-generation (measured: 2048-slot sections run at 2.6ns/slot
    # instead of 2.1).
    MAXB = 12 * P
    x = np.zeros_like(a)
    for g in range(ngroups):
        inc = np.zeros(n_cores, dtype=np.int64)
        for j in range(NCHUNKS - 1, 0, -1):
            eff = a[:, g, j] + inc
            room = np.maximum(MAXB - a[:, g, j - 1], 0)
            shed_max = np.minimum(f[:, g, j], room)
            cap = P * max(1, int(-(-(eff - shed_max).max() // P)))
            x[:, g, j] = np.clip(eff - cap, 0, shed_max)
            inc = x[:, g, j]
    # apply the moves: within each cell, flexible edges sort first, and
    # the first x of them drop to chunk j-1
    order_c = np.lexsort((~flex, cell))
    cs = cell[order_c]
    starts_c = np.searchsorted(cs, np.arange(ncell))
    move = np.zeros(rows.shape[0], dtype=bool)
    xf = x.reshape(-1)
    for ci in np.nonzero(xf)[0]:
        move[order_c[starts_c[ci] : starts_c[ci] + xf[ci]]] = True
    j_all = j_all - move.astype(np.int64)
    sec_all = grp_all * NCHUNKS + j_all

    # shared per-section block counts (max over cores)
    cnt = np.zeros((n_cores, nsec), dtype=np.int64)
    for c in range(n_cores):
        cnt[c] = np.bincount(sec_all[core == c], minlength=nsec)
    nblk = (cnt.max(axis=0) + P - 1) // P
    nblk = np.maximum(nblk, 1)
    blk_base = np.concatenate([[0], np.cumsum(nblk)[:-1]])
    NBg = int(nblk.sum())
    TOT = NBg * P

    # per-core slot packing + piece keys
    per_core_raw = []
    union_pk = set()
    for c in range(n_cores):
        m = core == c
        sc = sec_all[m]
        tc = t_all[m]
        dc = dit_all[m]
        vc = vals[m]
        colc = (cols[m] - j_all[m] * cc).astype(np.int16)
        o = np.lexsort((tc, sc))
        sc, tc, dc, vc, colc = sc[o], tc[o], dc[o], vc[o], colc[o]
        starts = np.concatenate([[0], np.cumsum(cnt[c])[:-1]])
        rank = np.arange(sc.size) - starts[sc]
        slot = blk_base[sc] * P + rank
        babs = blk_base[sc] + rank // P
        pk = babs * P + tc  # tile index < 128
        union_pk.update(np.unique(pk).tolist())
        per_core_raw.append((slot, babs, tc, dc, vc, colc, pk))

    # shared piece schedule in emission order (tile asc, then block asc)
    pk_u = np.array(sorted(union_pk), dtype=np.int64)
    babs_u = pk_u // P
    tl_u = pk_u % P
    order = np.lexsort((babs_u, tl_u))
    NP = pk_u.size
    mb_of_rank = np.empty(NP, dtype=np.int64)  # rank in pk_u -> mb
    mb_of_rank[order] = np.arange(NP)

    sec_of_blk = np.repeat(np.arange(nsec), nblk)
    pieces_mb_sorted = np.empty(NP, dtype=np.int64)
    pieces_mb_sorted[:] = np.arange(NP)
    # emission-order piece attributes
    e_babs = babs_u[order]
    e_tl = tl_u[order]
    e_j = sec_of_blk[e_babs] % NCHUNKS

    # per-tile first/last piece flags
    tile_first = np.zeros(NP, dtype=bool)
    tile_last = np.zeros(NP, dtype=bool)
    tile_first[0] = True
    for i in range(1, NP):
        if e_tl[i] != e_tl[i - 1]:
            tile_first[i] = True
            tile_last[i - 1] = True
    tile_last[NP - 1] = True

    groups = []
    for g in range(ngroups):
        g0 = int(g_bounds[g])
        g1 = int(g_bounds[g + 1])
        sections = []
        for j in range(NCHUNKS):
            s = g * NCHUNKS + j
            sections.append((int(blk_base[s]), int(nblk[s])))
        in_g = (e_tl >= g0) & (e_tl < g1)
        mbs = np.nonzero(in_g)[0]
        ms0, ms1 = int(mbs.min()), int(mbs.max()) + 1
        tile_pieces = []
        for t in range(g0, g1):
            sel = np.nonzero(e_tl == t)[0]
            tile_pieces.append(
                [
                    (int(mb), int(e_babs[mb]), int(e_j[mb]),
                     bool(tile_first[mb]), bool(tile_last[mb]))
                    for mb in sel
                ]
            )
        groups.append((g0, g1, sections, ms0, ms1, tile_pieces))

    # per-core payloads
    per_core = []
    for c in range(n_cores):
        slot, babs, tc, dc, vc, colc, pk = per_core_raw[c]
        # pad -> zero row at chunk-table index cc+HALO = 32767
        idx_flat = np.full(TOT, cc + HALO, dtype=np.int16)
        idx_flat[slot] = colc
        idx16 = np.tile(np.ascontiguousarray(idx_flat.reshape(-1, 16).T), (8, 1))
        # edge -> emission mb
        pos = np.searchsorted(pk_u, pk)
        mb_e = mb_of_rank[pos]
        S8 = np.zeros((P, NP, P), dtype=ml_dtypes.float8_e3m4)
        S8[slot % P, mb_e, dc] = vc.astype(np.float16)
        per_core.append(
            {
                "idx16": np.ascontiguousarray(idx16),
                "S8": np.ascontiguousarray(S8.reshape(P, NP * P)),
            }
        )

    sched = {
        "tiles": tiles,
        "npc": npc,
        "cc": cc,
        "halo": HALO,
        "groups": groups,
        "NBg": NBg,
        "TOT": TOT,
        "NP": NP,
    }
    return sched, per_core


def _raw_gather_128(eng, mybir, out_ap, in_ap, idxs_ap, num_idxs, queue_num,
                    num_idxs_reg=None):
    """dma_gather with a 128-byte element on a 256-byte-stride table.

    Mirrors bass's dma_gather (non-transpose, DRAM source, no prepare)
    but skips its 256B-element assert: the SWDGE ucode packetizes any
    elem_size (packet = min(elem_size_bytes, 16K)); only the row stride
    must be a 256B multiple (stride_bytes_256 field).
    """
    eng._assert_queue_num(queue_num)
    elem_size = 128  # fp8 elements = 128 bytes
    elem_step = 256  # table row stride in fp8 elements = 256 bytes
    assert in_ap.ap[0][0] == elem_step, in_ap.ap
    assert in_ap.ap[-1][1] == elem_size, in_ap.ap
    assert out_ap.ap[-1][1] == elem_size, out_ap.ap
    assert out_ap.ap[0][1] * out_ap.ap[1][1] == num_idxs, out_ap.ap
    _in_ap = eng.lower_ap_dma(in_ap, for_custom_bir_dma=True)
    _idxs_ap = eng.lower_ap(idxs_ap)
    _out_ap = eng.lower_ap(out_ap)
    return eng.add_instruction(
        mybir.InstDMAGatherAnt(
            name=eng.bass.get_next_instruction_name(),
            ins=[
                *_in_ap,
                _idxs_ap,
                eng.lower_val_access(
                    eng.to_reg(
                        num_idxs if num_idxs_reg is None else num_idxs_reg
                    )
                ),
            ],
            outs=[_out_ap],
            transpose=False,
            num_idxs=num_idxs,
            elem_size=elem_size,
            stride_bytes_256=1,
            gen_mode=0,
            single_packet=False,
            queue_num=queue_num,
            sbuf_tokens_per_rank=0,
            sbuf_free_dim_per_rank=0,
            sbuf_free_dim_pad_per_rank=0,
            sbuf_byte_offset=0,
        )
    )


def _build_program(n_nodes, sched):
    import concourse.bacc as bacc
    import concourse.mybir as mybir
    import concourse.tile as tile

    f32 = mybir.dt.float32
    f16 = mybir.dt.float16
    f8 = mybir.dt.float8e3
    i16 = mybir.dt.int16

    npc = sched["npc"]
    cc = sched["cc"]
    halo = sched["halo"]
    TOT = sched["TOT"]
    NP = sched["NP"]

    nc = bacc.Bacc(num_swdge_queues=4)
    feat8 = [
        nc.dram_tensor(f"feat8_{j}", [cc + halo + 1, 2, P], f8, kind="ExternalInput")
        for j in range(NCHUNKS)
    ]
    featT = nc.dram_tensor("featT", [D, npc], f16, kind="ExternalInput")
    w1 = nc.dram_tensor("W1", [D, D], f16, kind="ExternalInput")
    w2 = nc.dram_tensor("W2", [D, D], f16, kind="ExternalInput")
    bsum = nc.dram_tensor("bsum", [D, 2], f32, kind="ExternalInput")
    idx16 = nc.dram_tensor("idx16", [P, TOT // 16], i16, kind="ExternalInput")
    s8d = nc.dram_tensor("S8", [P, NP * P], f8, kind="ExternalInput")
    outT = nc.dram_tensor("outT", [D, npc], f16, kind="ExternalOutput")

    with tile.TileContext(nc) as tc:
        with (
            tc.tile_pool(name="const", bufs=1) as constp,
            tc.tile_pool(name="gpool", bufs=8) as gpool,
            tc.tile_pool(name="spool", bufs=3) as spool,
            tc.tile_pool(name="ftpool", bufs=4) as ftpool,
            tc.tile_pool(name="xspool", bufs=4) as xspool,
            tc.tile_pool(name="ampool", bufs=6) as ampool,
            tc.tile_pool(name="ostage", bufs=3) as ostagep,
            tc.tile_pool(name="psx", bufs=6, space="PSUM") as psx,
            tc.tile_pool(name="pso", bufs=2, space="PSUM") as pso,
        ):
            # --- constants.  idx16 is loaded in per-group slices, two
            # groups ahead of use, so no gather ever waits on a bulk
            # index transfer (an 11.9us pipeline bubble otherwise).
            grp_rng = [
                (g[2][0][0], g[2][-1][0] + g[2][-1][1]) for g in sched["groups"]
            ]
            idx16_t = constp.tile([P, TOT // 16], i16)

            def load_idx_slice(g):
                b0, b1 = grp_rng[g]
                nc.scalar.dma_start(
                    out=idx16_t[:, b0 * 8 : b1 * 8], in_=idx16[:, b0 * 8 : b1 * 8]
                )

            load_idx_slice(0)
            load_idx_slice(1)
            w1_t = constp.tile([P, P], f16)
            nc.sync.dma_start(out=w1_t[:], in_=w1[:, :])
            w2_t = constp.tile([P, P], f16)
            nc.sync.dma_start(out=w2_t[:], in_=w2[:, :])
            bias_t = constp.tile([P, 2], f32)
            nc.sync.dma_start(out=bias_t[:], in_=bsum[:, :])

            def emit_epilogue(g0, gw, fT, xS, oT):
                aT = ampool.tile([P, gw], f16, tag="aT")
                mT = ampool.tile([P, gw], f16, tag="mT")
                nc.vector.tensor_tensor(
                    out=aT[:], in0=xS[:, :gw], in1=fT[:, :gw],
                    op=mybir.AluOpType.add,
                )
                nc.vector.tensor_tensor(
                    out=mT[:], in0=xS[:, :gw], in1=fT[:, :gw],
                    op=mybir.AluOpType.mult,
                )
                for c0 in range(0, gw, 512):
                    cw = min(512, gw - c0)
                    out2 = pso.tile([P, 512], f32, tag="out2")
                    nc.tensor.matmul(
                        out=out2[:, :cw], lhsT=w1_t[:], rhs=aT[:, c0 : c0 + cw],
                        start=True, stop=False,
                    )
                    nc.tensor.matmul(
                        out=out2[:, :cw], lhsT=w2_t[:], rhs=mT[:, c0 : c0 + cw],
                        start=False, stop=True,
                    )
                    nc.scalar.activation(
                        out=oT[:, c0 : c0 + cw],
                        in_=out2[:, :cw],
                        func=mybir.ActivationFunctionType.Identity,
                        bias=bias_t[:, 0:1],
                        scale=1.0,
                    )
                nc.scalar.dma_start(
                    out=outT[:, g0 * P : g0 * P + gw], in_=oT[:, :gw]
                )

            prev = None
            for gi, (g0, g1, sections, ms0, ms1, tile_pieces) in enumerate(
                sched["groups"]
            ):
                gw = min(g1 * P, npc) - g0 * P
                npg = ms1 - ms0
                if gi + 2 < len(sched["groups"]):
                    load_idx_slice(gi + 2)

                # one dma_gather per feature-table chunk, parallel SWDGE
                # queues; 128B fp8 payload per edge from 256B-stride rows
                gtiles = {}
                for j in range(NCHUNKS):
                    sec_start, sec_nblk = sections[j]
                    G = gpool.tile([P, sec_nblk, P], f8, tag=f"G{j}")
                    n_idx = sec_nblk * P
                    _raw_gather_128(
                        nc.gpsimd,
                        mybir,
                        G[:],
                        feat8[j][:, 0, :],
                        idx16_t[:, sec_start * 8 : sec_start * 8 + n_idx // 16],
                        n_idx,
                        queue_num=j,
                    )
                    gtiles[j] = (G, sec_start)

                # host-built one-hot*val S blocks, streamed on SP ring
                S = spool.tile([P, npg, P], f8, tag="S")
                nc.sync.dma_start(out=S[:], in_=s8d[:, ms0 * P : ms1 * P])

                # featT slice for this group, on the ACT HWDGE ring
                fT = ftpool.tile([P, gw], f16, tag="fT")
                nc.scalar.dma_start(out=fT[:], in_=featT[:, g0 * P : g0 * P + gw])

                # dense epilogue for the PREVIOUS group, emitted BEFORE this
                # group's segment matmuls: its inputs (xS/fT of g-1) are
                # ready, so the PE spends the wait for this group's gather
                # drain on epilogue work instead of idling, and the
                # post-final-gather tail shrinks by one group's epilogue.
                # (The old after-matmuls ordering existed for the on-chip
                # DVE S-build, which is gone.)
                if prev is not None:
                    emit_epilogue(*prev)

                # xT psum per tile; evict to fp16 SBUF on the scalar engine
                xS = xspool.tile([P, gw], f16, tag="xS")
                for t in range(g0, g1):
                    w = min((t + 1) * P, npc) - t * P
                    pieces = tile_pieces[t - g0]
                    xT = psx.tile([P, P], f32, tag="xT")
                    for mb, babs, j, st, sp in pieces:
                        G, sec_start = gtiles[j]
                        nc.tensor.matmul(
                            out=xT[:],
                            lhsT=G[:, babs - sec_start, :],
                            rhs=S[:, mb - ms0, :],
                            start=st,
                            stop=sp,
                        )
                    toff = (t - g0) * P
                    nc.scalar.activation(
                        out=xS[:, toff : toff + w],
                        in_=xT[:, :w],
                        func=mybir.ActivationFunctionType.Identity,
                        bias=bias_t[:, 1:2],
                        scale=1.0,
                    )

                oT = ostagep.tile([P, gw], f16, tag="oT")
                prev = (g0, gw, fT, xS, oT)

            emit_epilogue(*prev)
    nc.compile()
    return nc


def _run(rows, cols, vals, features, W1, b1, W2, b2, n_nodes, n_cores):
    global _LAST_RESULTS
    import ml_dtypes
    from concourse import bass_utils

    npc = n_nodes // n_cores
    features = np.ascontiguousarray(np.asarray(features, dtype=np.float32))
    W1_16 = np.ascontiguousarray(np.asarray(W1, dtype=np.float32).astype(np.float16))
    W2_16 = np.ascontiguousarray(np.asarray(W2, dtype=np.float32).astype(np.float16))
    bsum = np.zeros((D, 2), dtype=np.float32)
    bsum[:, 0] = np.asarray(b1, dtype=np.float32) + np.asarray(b2, dtype=np.float32)
    bsum = np.ascontiguousarray(bsum)

    sched, per_core = _prep(rows, cols, vals, n_nodes, n_cores)
    nc = _build_program(n_nodes, sched)

    cc = sched["cc"]
    halo = sched["halo"]
    feat8_flat = features.astype(np.float16).astype(ml_dtypes.float8_e3m4)
    # halo-extended chunks: chunk j covers table rows [j*cc, j*cc+cc+halo)
    # so down-shifted edges (idx in [cc, cc+halo)) resolve in-bounds; the
    # final row (index cc+halo = 32767) stays zero as the padding target.
    tabpad = np.zeros((N_NODES + halo + 1, P), dtype=ml_dtypes.float8_e3m4)
    tabpad[:N_NODES] = feat8_flat
    feat8_chunks = []
    for j in range(NCHUNKS):
        chunk = np.zeros((cc + halo + 1, 2, P), dtype=ml_dtypes.float8_e3m4)
        chunk[: cc + halo, 0, :] = tabpad[j * cc : j * cc + cc + halo]
        feat8_chunks.append(np.ascontiguousarray(chunk))

    in_maps = []
    for c in range(n_cores):
        featT_c = np.ascontiguousarray(
            features[c * npc : (c + 1) * npc, :].T.astype(np.float16)
        )
        im = {
            "featT": featT_c,
            "W1": W1_16,
            "W2": W2_16,
            "bsum": bsum,
            "idx16": per_core[c]["idx16"],
            "S8": per_core[c]["S8"],
        }
        for j in range(NCHUNKS):
            im[f"feat8_{j}"] = feat8_chunks[j]
        in_maps.append(im)

    res = bass_utils.run_bass_kernel_spmd(nc, in_maps, core_ids=list(range(n_cores)))
    _LAST_RESULTS = res
    out = np.concatenate(
        [r["outT"].T.astype(np.float32) for r in res.results], axis=0
    )
    return np.ascontiguousarray(out)


def kernel(rows, cols, vals, features, W1, b1, W2, b2):
    return _run(rows, cols, vals, features, W1, b1, W2, b2, N_NODES, N_CORES)


# revision 47
# speedup vs baseline: 1.1247x; 1.1247x over previous
"""BiGNN message-passing kernel for Trainium2 (8 NeuronCores, Bass/Tile).

Reference computation (N=100000 nodes, E=600000 edges, D=128):
    msgs = vals[:, None] * features[cols]            # gather + scale
    x    = segment_sum(msgs, rows)                   # scatter-add to rows
    out  = (features + x) @ W1 + b1 + (x * features) @ W2 + b2

Sharding: destination nodes (rows) are sharded across the 8 cores, 12500
each; `features` is replicated into every core's HBM, so the per-edge
source gather is core-local (no collectives).

The critical path is GPSIMD (SWDGE) descriptor generation for the
per-edge feature gather: ~2.3 ns/index, strictly serialized on the one
POOL engine (each InstDMAGatherAnt activates only the Q7 core pair of
its queue).  Everything else is arranged to hide underneath it:

  * gathered source features G: fp8e3m4 table laid out [cc+1, 2, 128]
    (payload in [:, 0, :], row cc all-zero for padding slots), gathered
    as 128B elements from 256B-stride rows via a raw InstDMAGatherAnt.
  * edge slots are packed DENSELY per (group, chunk) section, sorted by
    destination tile: blocks of 128 slots may straddle tile boundaries,
    cutting gather padding from 25% to ~8% (descgen is ~2.1ns/slot, so
    padding is pure critical-path loss).  The matmul schedule is the
    UNION over the 8 cores of (block, tile) pieces; a core lacking a
    piece gets an all-zero S block there (contributes nothing).
  * the one-hot scatter matrices S (S[slot, dst] = val, one 128x128 fp8
    block per piece) are built on the HOST and STREAMED from HBM on the
    SP HWDGE ring instead of being built on DVE (which used to be a
    second ~235us serial bottleneck contending with GPSIMD for the
    shared POOL SBUF port).
  * idx16 loads in per-group slices two groups ahead, so no gather ever
    waits on a bulk index transfer; the group schedule tapers to
    [4,3,2,1] tiles at the end so the post-descgen PE/epilogue tail is
    a few us instead of ~20.

Rejected variants (measured slower): building S on DVE (the original
281us baseline); 12/16-tile groups (S/G buffer pressure); trailing
negative-index trimming with per-core counts in num_idxs_reg — correct,
but the ucode's scalar trailing-scan costs more than the skipped pad
blocks save (231us), and with the count left at the padded value the
decode-side ring reservation mismatches the written descriptors and
wedges the device.

The segment-sum runs on TensorE, one matmul per piece:

    xT[f, d] += G[e, f].T @ S[e, d]        (fp8 x fp8 -> f32 psum)

Finished xT psums are evicted to SBUF in fp16 by the scalar engine, and
the dense epilogue for group g-1 is emitted inside group g:

    outT = W1.T @ (fT + xT) + W2.T @ (xT * fT) + (b1 + b2)

featT / outT move in fp16 on the ACT ring; the host transposes and
upcasts per-core outputs back to fp32.
"""

import numpy as np

P = 128
D = 128
N_NODES = 100000
N_EDGES = 600000
N_CORES = 8
NCHUNKS = 4  # feature-table column chunks (int16 index reach)
# dest tiles per gather/store group: small first groups fill the pipeline
# fast, small last groups shrink the post-descgen tail (the PE/epilogue
# work that serializes after the final gather)
GROUP_SIZES = [8] * 11 + [6, 4]

_LAST_RESULTS = None  # BassKernelResults of the most recent run (for test.py)


def _prep(rows, cols, vals, n_nodes, n_cores):
    """Host-side edge reorganization into the shared block schedule.

    Returns (sched, per_core):
      sched:
        tiles/npc/cc/ngroups/TOT/NP plus per-group gather sections and
        the shared matmul piece schedule (union over cores).
      per_core[c]:
        idx16 [128, TOT/16] int16   gather indices (pad -> zero row cc)
        S8    [128, NP*128] fp8e3m4 one-hot*val scatter blocks
    """
    import ml_dtypes

    npc = n_nodes // n_cores
    tiles = (npc + P - 1) // P
    assert sum(GROUP_SIZES) == tiles, (sum(GROUP_SIZES), tiles)
    ngroups = len(GROUP_SIZES)
    nsec = ngroups * NCHUNKS
    cc = n_nodes // NCHUNKS
    assert n_nodes % NCHUNKS == 0
    g_bounds = np.concatenate([[0], np.cumsum(GROUP_SIZES)])
    group_of_tile = np.repeat(np.arange(ngroups), GROUP_SIZES)

    rows = np.asarray(rows, dtype=np.int64)
    cols = np.asarray(cols, dtype=np.int64)
    vals = np.asarray(vals, dtype=np.float32)

    core = rows // npc
    local = rows - core * npc
    t_all = local // P
    dit_all = (local - t_all * P).astype(np.int64)
    j_all = (cols // cc).astype(np.int64)

    sec_all = group_of_tile[t_all] * NCHUNKS + j_all

    # shared per-section block counts (max over cores)
    cnt = np.zeros((n_cores, nsec), dtype=np.int64)
    for c in range(n_cores):
        cnt[c] = np.bincount(sec_all[core == c], minlength=nsec)
    nblk = (cnt.max(axis=0) + P - 1) // P
    nblk = np.maximum(nblk, 1)
    blk_base = np.concatenate([[0], np.cumsum(nblk)[:-1]])
    NBg = int(nblk.sum())
    TOT = NBg * P

    # per-core slot packing + piece keys
    per_core_raw = []
    union_pk = set()
    for c in range(n_cores):
        m = core == c
        sc = sec_all[m]
        tc = t_all[m]
        dc = dit_all[m]
        vc = vals[m]
        colc = (cols[m] - j_all[m] * cc).astype(np.int16)
        o = np.lexsort((tc, sc))
        sc, tc, dc, vc, colc = sc[o], tc[o], dc[o], vc[o], colc[o]
        starts = np.concatenate([[0], np.cumsum(cnt[c])[:-1]])
        rank = np.arange(sc.size) - starts[sc]
        slot = blk_base[sc] * P + rank
        babs = blk_base[sc] + rank // P
        pk = babs * P + tc  # tile index < 128
        union_pk.update(np.unique(pk).tolist())
        per_core_raw.append((slot, babs, tc, dc, vc, colc, pk))

    # shared piece schedule in emission order (tile asc, then block asc)
    pk_u = np.array(sorted(union_pk), dtype=np.int64)
    babs_u = pk_u // P
    tl_u = pk_u % P
    order = np.lexsort((babs_u, tl_u))
    NP = pk_u.size
    mb_of_rank = np.empty(NP, dtype=np.int64)  # rank in pk_u -> mb
    mb_of_rank[order] = np.arange(NP)

    sec_of_blk = np.repeat(np.arange(nsec), nblk)
    pieces_mb_sorted = np.empty(NP, dtype=np.int64)
    pieces_mb_sorted[:] = np.arange(NP)
    # emission-order piece attributes
    e_babs = babs_u[order]
    e_tl = tl_u[order]
    e_j = sec_of_blk[e_babs] % NCHUNKS

    # per-tile first/last piece flags
    tile_first = np.zeros(NP, dtype=bool)
    tile_last = np.zeros(NP, dtype=bool)
    tile_first[0] = True
    for i in range(1, NP):
        if e_tl[i] != e_tl[i - 1]:
            tile_first[i] = True
            tile_last[i - 1] = True
    tile_last[NP - 1] = True

    groups = []
    for g in range(ngroups):
        g0 = int(g_bounds[g])
        g1 = int(g_bounds[g + 1])
        sections = []
        for j in range(NCHUNKS):
            s = g * NCHUNKS + j
            sections.append((int(blk_base[s]), int(nblk[s])))
        in_g = (e_tl >= g0) & (e_tl < g1)
        mbs = np.nonzero(in_g)[0]
        ms0, ms1 = int(mbs.min()), int(mbs.max()) + 1
        tile_pieces = []
        for t in range(g0, g1):
            sel = np.nonzero(e_tl == t)[0]
            tile_pieces.append(
                [
                    (int(mb), int(e_babs[mb]), int(e_j[mb]),
                     bool(tile_first[mb]), bool(tile_last[mb]))
                    for mb in sel
                ]
            )
        groups.append((g0, g1, sections, ms0, ms1, tile_pieces))

    # per-core payloads
    per_core = []
    for c in range(n_cores):
        slot, babs, tc, dc, vc, colc, pk = per_core_raw[c]
        idx_flat = np.full(TOT, cc, dtype=np.int16)  # pad -> zero row
        idx_flat[slot] = colc
        idx16 = np.tile(np.ascontiguousarray(idx_flat.reshape(-1, 16).T), (8, 1))
        # edge -> emission mb
        pos = np.searchsorted(pk_u, pk)
        mb_e = mb_of_rank[pos]
        S8 = np.zeros((P, NP, P), dtype=ml_dtypes.float8_e3m4)
        S8[slot % P, mb_e, dc] = vc.astype(np.float16)
        per_core.append(
            {
                "idx16": np.ascontiguousarray(idx16),
                "S8": np.ascontiguousarray(S8.reshape(P, NP * P)),
            }
        )

    sched = {
        "tiles": tiles,
        "npc": npc,
        "cc": cc,
        "groups": groups,
        "NBg": NBg,
        "TOT": TOT,
        "NP": NP,
    }
    return sched, per_core


def _raw_gather_128(eng, mybir, out_ap, in_ap, idxs_ap, num_idxs, queue_num,
                    num_idxs_reg=None):
    """dma_gather with a 128-byte element on a 256-byte-stride table.

    Mirrors bass's dma_gather (non-transpose, DRAM source, no prepare)
    but skips its 256B-element assert: the SWDGE ucode packetizes any
    elem_size (packet = min(elem_size_bytes, 16K)); only the row stride
    must be a 256B multiple (stride_bytes_256 field).
    """
    eng._assert_queue_num(queue_num)
    elem_size = 128  # fp8 elements = 128 bytes
    elem_step = 256  # table row stride in fp8 elements = 256 bytes
    assert in_ap.ap[0][0] == elem_step, in_ap.ap
    assert in_ap.ap[-1][1] == elem_size, in_ap.ap
    assert out_ap.ap[-1][1] == elem_size, out_ap.ap
    assert out_ap.ap[0][1] * out_ap.ap[1][1] == num_idxs, out_ap.ap
    _in_ap = eng.lower_ap_dma(in_ap, for_custom_bir_dma=True)
    _idxs_ap = eng.lower_ap(idxs_ap)
    _out_ap = eng.lower_ap(out_ap)
    return eng.add_instruction(
        mybir.InstDMAGatherAnt(
            name=eng.bass.get_next_instruction_name(),
            ins=[
                *_in_ap,
                _idxs_ap,
                eng.lower_val_access(
                    eng.to_reg(
                        num_idxs if num_idxs_reg is None else num_idxs_reg
                    )
                ),
            ],
            outs=[_out_ap],
            transpose=False,
            num_idxs=num_idxs,
            elem_size=elem_size,
            stride_bytes_256=1,
            gen_mode=0,
            single_packet=False,
            queue_num=queue_num,
            sbuf_tokens_per_rank=0,
            sbuf_free_dim_per_rank=0,
            sbuf_free_dim_pad_per_rank=0,
            sbuf_byte_offset=0,
        )
    )


def _build_program(n_nodes, sched):
    import concourse.bacc as bacc
    import concourse.mybir as mybir
    import concourse.tile as tile

    f32 = mybir.dt.float32
    f16 = mybir.dt.float16
    f8 = mybir.dt.float8e3
    i16 = mybir.dt.int16

    npc = sched["npc"]
    cc = sched["cc"]
    TOT = sched["TOT"]
    NP = sched["NP"]

    nc = bacc.Bacc(num_swdge_queues=4)
    feat8 = [
        nc.dram_tensor(f"feat8_{j}", [cc + 1, 2, P], f8, kind="ExternalInput")
        for j in range(NCHUNKS)
    ]
    featT = nc.dram_tensor("featT", [D, npc], f16, kind="ExternalInput")
    w1 = nc.dram_tensor("W1", [D, D], f16, kind="ExternalInput")
    w2 = nc.dram_tensor("W2", [D, D], f16, kind="ExternalInput")
    bsum = nc.dram_tensor("bsum", [D, 2], f32, kind="ExternalInput")
    idx16 = nc.dram_tensor("idx16", [P, TOT // 16], i16, kind="ExternalInput")
    s8d = nc.dram_tensor("S8", [P, NP * P], f8, kind="ExternalInput")
    outT = nc.dram_tensor("outT", [D, npc], f16, kind="ExternalOutput")

    with tile.TileContext(nc) as tc:
        with (
            tc.tile_pool(name="const", bufs=1) as constp,
            tc.tile_pool(name="gpool", bufs=8) as gpool,
            tc.tile_pool(name="spool", bufs=3) as spool,
            tc.tile_pool(name="ftpool", bufs=4) as ftpool,
            tc.tile_pool(name="xspool", bufs=4) as xspool,
            tc.tile_pool(name="ampool", bufs=6) as ampool,
            tc.tile_pool(name="ostage", bufs=3) as ostagep,
            tc.tile_pool(name="psx", bufs=6, space="PSUM") as psx,
            tc.tile_pool(name="pso", bufs=2, space="PSUM") as pso,
        ):
            # --- constants.  idx16 is loaded in per-group slices, two
            # groups ahead of use, so no gather ever waits on a bulk
            # index transfer (an 11.9us pipeline bubble otherwise).
            grp_rng = [
                (g[2][0][0], g[2][-1][0] + g[2][-1][1]) for g in sched["groups"]
            ]
            idx16_t = constp.tile([P, TOT // 16], i16)

            def load_idx_slice(g):
                b0, b1 = grp_rng[g]
                nc.scalar.dma_start(
                    out=idx16_t[:, b0 * 8 : b1 * 8], in_=idx16[:, b0 * 8 : b1 * 8]
                )

            load_idx_slice(0)
            load_idx_slice(1)
            w1_t = constp.tile([P, P], f16)
            nc.sync.dma_start(out=w1_t[:], in_=w1[:, :])
            w2_t = constp.tile([P, P], f16)
            nc.sync.dma_start(out=w2_t[:], in_=w2[:, :])
            bias_t = constp.tile([P, 2], f32)
            nc.sync.dma_start(out=bias_t[:], in_=bsum[:, :])

            def emit_epilogue(g0, gw, fT, xS, oT):
                aT = ampool.tile([P, gw], f16, tag="aT")
                mT = ampool.tile([P, gw], f16, tag="mT")
                nc.vector.tensor_tensor(
                    out=aT[:], in0=xS[:, :gw], in1=fT[:, :gw],
                    op=mybir.AluOpType.add,
                )
                nc.vector.tensor_tensor(
                    out=mT[:], in0=xS[:, :gw], in1=fT[:, :gw],
                    op=mybir.AluOpType.mult,
                )
                for c0 in range(0, gw, 512):
                    cw = min(512, gw - c0)
                    out2 = pso.tile([P, 512], f32, tag="out2")
                    nc.tensor.matmul(
                        out=out2[:, :cw], lhsT=w1_t[:], rhs=aT[:, c0 : c0 + cw],
                        start=True, stop=False,
                    )
                    nc.tensor.matmul(
                        out=out2[:, :cw], lhsT=w2_t[:], rhs=mT[:, c0 : c0 + cw],
                        start=False, stop=True,
                    )
                    nc.scalar.activation(
                        out=oT[:, c0 : c0 + cw],
                        in_=out2[:, :cw],
                        func=mybir.ActivationFunctionType.Identity,
                        bias=bias_t[:, 0:1],
                        scale=1.0,
                    )
                nc.scalar.dma_start(
                    out=outT[:, g0 * P : g0 * P + gw], in_=oT[:, :gw]
                )

            prev = None
            for gi, (g0, g1, sections, ms0, ms1, tile_pieces) in enumerate(
                sched["groups"]
            ):
                gw = min(g1 * P, npc) - g0 * P
                npg = ms1 - ms0
                if gi + 2 < len(sched["groups"]):
                    load_idx_slice(gi + 2)

                # one dma_gather per feature-table chunk, parallel SWDGE
                # queues; 128B fp8 payload per edge from 256B-stride rows
                gtiles = {}
                for j in range(NCHUNKS):
                    sec_start, sec_nblk = sections[j]
                    G = gpool.tile([P, sec_nblk, P], f8, tag=f"G{j}")
                    n_idx = sec_nblk * P
                    _raw_gather_128(
                        nc.gpsimd,
                        mybir,
                        G[:],
                        feat8[j][:, 0, :],
                        idx16_t[:, sec_start * 8 : sec_start * 8 + n_idx // 16],
                        n_idx,
                        queue_num=j,
                    )
                    gtiles[j] = (G, sec_start)

                # host-built one-hot*val S blocks, streamed on SP ring
                S = spool.tile([P, npg, P], f8, tag="S")
                nc.sync.dma_start(out=S[:], in_=s8d[:, ms0 * P : ms1 * P])

                # featT slice for this group, on the ACT HWDGE ring
                fT = ftpool.tile([P, gw], f16, tag="fT")
                nc.scalar.dma_start(out=fT[:], in_=featT[:, g0 * P : g0 * P + gw])

                # dense epilogue for the PREVIOUS group, emitted BEFORE this
                # group's segment matmuls: its inputs (xS/fT of g-1) are
                # ready, so the PE spends the wait for this group's gather
                # drain on epilogue work instead of idling, and the
                # post-final-gather tail shrinks by one group's epilogue.
                # (The old after-matmuls ordering existed for the on-chip
                # DVE S-build, which is gone.)
                if prev is not None:
                    emit_epilogue(*prev)

                # xT psum per tile; evict to fp16 SBUF on the scalar engine
                xS = xspool.tile([P, gw], f16, tag="xS")
                for t in range(g0, g1):
                    w = min((t + 1) * P, npc) - t * P
                    pieces = tile_pieces[t - g0]
                    xT = psx.tile([P, P], f32, tag="xT")
                    for mb, babs, j, st, sp in pieces:
                        G, sec_start = gtiles[j]
                        nc.tensor.matmul(
                            out=xT[:],
                            lhsT=G[:, babs - sec_start, :],
                            rhs=S[:, mb - ms0, :],
                            start=st,
                            stop=sp,
                        )
                    toff = (t - g0) * P
                    nc.scalar.activation(
                        out=xS[:, toff : toff + w],
                        in_=xT[:, :w],
                        func=mybir.ActivationFunctionType.Identity,
                        bias=bias_t[:, 1:2],
                        scale=1.0,
                    )

                oT = ostagep.tile([P, gw], f16, tag="oT")
                prev = (g0, gw, fT, xS, oT)

            emit_epilogue(*prev)
    nc.compile()
    return nc


def _run(rows, cols, vals, features, W1, b1, W2, b2, n_nodes, n_cores):
    global _LAST_RESULTS
    import ml_dtypes
    from concourse import bass_utils

    npc = n_nodes // n_cores
    features = np.ascontiguousarray(np.asarray(features, dtype=np.float32))
    W1_16 = np.ascontiguousarray(np.asarray(W1, dtype=np.float32).astype(np.float16))
    W2_16 = np.ascontiguousarray(np.asarray(W2, dtype=np.float32).astype(np.float16))
    bsum = np.zeros((D, 2), dtype=np.float32)
    bsum[:, 0] = np.asarray(b1, dtype=np.float32) + np.asarray(b2, dtype=np.float32)
    bsum = np.ascontiguousarray(bsum)

    sched, per_core = _prep(rows, cols, vals, n_nodes, n_cores)
    nc = _build_program(n_nodes, sched)

    cc = sched["cc"]
    feat8_flat = features.astype(np.float16).astype(ml_dtypes.float8_e3m4)
    feat8_chunks = []
    for j in range(NCHUNKS):
        chunk = np.zeros((cc + 1, 2, P), dtype=ml_dtypes.float8_e3m4)
        chunk[:cc, 0, :] = feat8_flat[j * cc : (j + 1) * cc, :]
        feat8_chunks.append(np.ascontiguousarray(chunk))

    in_maps = []
    for c in range(n_cores):
        featT_c = np.ascontiguousarray(
            features[c * npc : (c + 1) * npc, :].T.astype(np.float16)
        )
        im = {
            "featT": featT_c,
            "W1": W1_16,
            "W2": W2_16,
            "bsum": bsum,
            "idx16": per_core[c]["idx16"],
            "S8": per_core[c]["S8"],
        }
        for j in range(NCHUNKS):
            im[f"feat8_{j}"] = feat8_chunks[j]
        in_maps.append(im)

    res = bass_utils.run_bass_kernel_spmd(nc, in_maps, core_ids=list(range(n_cores)))
    _LAST_RESULTS = res
    out = np.concatenate(
        [r["outT"].T.astype(np.float32) for r in res.results], axis=0
    )
    return np.ascontiguousarray(out)


def kernel(rows, cols, vals, features, W1, b1, W2, b2):
    return _run(rows, cols, vals, features, W1, b1, W2, b2, N_NODES, N_CORES)


# revision 51
# speedup vs baseline: 1.1422x; 1.0155x over previous
"""BiGNN message-passing kernel for Trainium2 (8 NeuronCores, Bass/Tile).

Reference computation (N=100000 nodes, E=600000 edges, D=128):
    msgs = vals[:, None] * features[cols]            # gather + scale
    x    = segment_sum(msgs, rows)                   # scatter-add to rows
    out  = (features + x) @ W1 + b1 + (x * features) @ W2 + b2

Sharding: destination nodes (rows) are sharded across the 8 cores, 12500
each; `features` is replicated into every core's HBM, so the per-edge
source gather is core-local (no collectives).

The critical path is GPSIMD (SWDGE) descriptor generation for the
per-edge feature gather: ~2.3 ns/index, strictly serialized on the one
POOL engine (each InstDMAGatherAnt activates only the Q7 core pair of
its queue).  Everything else is arranged to hide underneath it:

  * gathered source features G: fp8e3m4 table laid out [cc+1, 2, 128]
    (payload in [:, 0, :], row cc all-zero for padding slots), gathered
    as 128B elements from 256B-stride rows via a raw InstDMAGatherAnt.
  * edge slots are packed DENSELY per (group, chunk) section, sorted by
    destination tile: blocks of 128 slots may straddle tile boundaries,
    cutting gather padding from 25% to ~8% (descgen is ~2.1ns/slot, so
    padding is pure critical-path loss).  The matmul schedule is the
    UNION over the 8 cores of (block, tile) pieces; a core lacking a
    piece gets an all-zero S block there (contributes nothing).
  * the one-hot scatter matrices S (S[slot, dst] = val, one 128x128 fp8
    block per piece) are built on the HOST and STREAMED from HBM on the
    SP HWDGE ring instead of being built on DVE (which used to be a
    second ~235us serial bottleneck contending with GPSIMD for the
    shared POOL SBUF port).
  * idx16 loads in per-group slices two groups ahead, so no gather ever
    waits on a bulk index transfer; the group schedule tapers to
    [4,3,2,1] tiles at the end so the post-descgen PE/epilogue tail is
    a few us instead of ~20.

Measured hardware model (core 0, 8 cores SPMD; best run 204504 ns):
  ~21us  fixed startup: framework preamble + two invisible GPSIMD ucode
         IRAM loads before the first gather can decode.
  ~173us descgen: 82432 slots x 1.85 ns/slot + ~335 ns fixed per gather
         instruction, gap-free and strictly serial on the POOL engine.
  ~11us  tail: leftover PE pieces of the tapered groups (~107 ns each,
         LDWEIGHTS-serialized) + final store/teardown.
Sections are capped at 13 blocks (~1664 slots): the decode reserves the
whole instruction's descriptors in the per-queue SWDGE ring up front,
and 16+-block sections measurably stall descgen (2.6+ ns/slot).

Rejected variants (all measured slower): building S on DVE (the
original 281us baseline); 12/16-tile groups and [6,4] tails (ring
overflow / fatter tail); trailing negative-index trimming with per-core
counts in num_idxs_reg — correct, but the ucode's scalar trailing-scan
costs more than the skipped pad blocks save (231us), and with the count
left at the padded value the decode-side ring reservation mismatches
the written descriptors and wedges the device; halo-overlapped chunk
tables that rebalance edges between adjacent sections (TOT -6%, but
per-slot descgen rose 6-25% with reshaped sections, netting zero to
-35us).

The segment-sum runs on TensorE, one matmul per piece:

    xT[f, d] += G[e, f].T @ S[e, d]        (fp8 x fp8 -> f32 psum)

Finished xT psums are evicted to SBUF in fp16 by the scalar engine, and
the dense epilogue for group g-1 is emitted inside group g:

    outT = W1.T @ (fT + xT) + W2.T @ (xT * fT) + (b1 + b2)

featT / outT move in fp16 on the ACT ring; the host transposes and
upcasts per-core outputs back to fp32.
"""

import numpy as np

P = 128
D = 128
N_NODES = 100000
N_EDGES = 600000
N_CORES = 8
NCHUNKS = 4  # feature-table column chunks (int16 index reach)
# dest tiles per gather/store group: small first groups fill the pipeline
# fast, small last groups shrink the post-descgen tail (the PE/epilogue
# work that serializes after the final gather)
GROUP_SIZES = [8] * 11 + [4, 3, 2, 1]

_LAST_RESULTS = None  # BassKernelResults of the most recent run (for test.py)


def _prep(rows, cols, vals, n_nodes, n_cores):
    """Host-side edge reorganization into the shared block schedule.

    Returns (sched, per_core):
      sched:
        tiles/npc/cc/ngroups/TOT/NP plus per-group gather sections and
        the shared matmul piece schedule (union over cores).
      per_core[c]:
        idx16 [128, TOT/16] int16   gather indices (pad -> zero row cc)
        S8    [128, NP*128] fp8e3m4 one-hot*val scatter blocks
    """
    import ml_dtypes

    npc = n_nodes // n_cores
    tiles = (npc + P - 1) // P
    assert sum(GROUP_SIZES) == tiles, (sum(GROUP_SIZES), tiles)
    ngroups = len(GROUP_SIZES)
    nsec = ngroups * NCHUNKS
    cc = n_nodes // NCHUNKS
    assert n_nodes % NCHUNKS == 0
    g_bounds = np.concatenate([[0], np.cumsum(GROUP_SIZES)])
    group_of_tile = np.repeat(np.arange(ngroups), GROUP_SIZES)

    rows = np.asarray(rows, dtype=np.int64)
    cols = np.asarray(cols, dtype=np.int64)
    vals = np.asarray(vals, dtype=np.float32)

    core = rows // npc
    local = rows - core * npc
    t_all = local // P
    dit_all = (local - t_all * P).astype(np.int64)
    j_all = (cols // cc).astype(np.int64)

    sec_all = group_of_tile[t_all] * NCHUNKS + j_all

    # shared per-section block counts (max over cores)
    cnt = np.zeros((n_cores, nsec), dtype=np.int64)
    for c in range(n_cores):
        cnt[c] = np.bincount(sec_all[core == c], minlength=nsec)
    nblk = (cnt.max(axis=0) + P - 1) // P
    nblk = np.maximum(nblk, 1)
    blk_base = np.concatenate([[0], np.cumsum(nblk)[:-1]])
    NBg = int(nblk.sum())
    TOT = NBg * P

    # per-core slot packing + piece keys
    per_core_raw = []
    union_pk = set()
    for c in range(n_cores):
        m = core == c
        sc = sec_all[m]
        tc = t_all[m]
        dc = dit_all[m]
        vc = vals[m]
        colc = (cols[m] - j_all[m] * cc).astype(np.int16)
        o = np.lexsort((tc, sc))
        sc, tc, dc, vc, colc = sc[o], tc[o], dc[o], vc[o], colc[o]
        starts = np.concatenate([[0], np.cumsum(cnt[c])[:-1]])
        rank = np.arange(sc.size) - starts[sc]
        slot = blk_base[sc] * P + rank
        babs = blk_base[sc] + rank // P
        pk = babs * P + tc  # tile index < 128
        union_pk.update(np.unique(pk).tolist())
        per_core_raw.append((slot, babs, tc, dc, vc, colc, pk))

    # shared piece schedule in emission order (tile asc, then block asc)
    pk_u = np.array(sorted(union_pk), dtype=np.int64)
    babs_u = pk_u // P
    tl_u = pk_u % P
    order = np.lexsort((babs_u, tl_u))
    NP = pk_u.size
    mb_of_rank = np.empty(NP, dtype=np.int64)  # rank in pk_u -> mb
    mb_of_rank[order] = np.arange(NP)

    sec_of_blk = np.repeat(np.arange(nsec), nblk)
    pieces_mb_sorted = np.empty(NP, dtype=np.int64)
    pieces_mb_sorted[:] = np.arange(NP)
    # emission-order piece attributes
    e_babs = babs_u[order]
    e_tl = tl_u[order]
    e_j = sec_of_blk[e_babs] % NCHUNKS

    # per-tile first/last piece flags
    tile_first = np.zeros(NP, dtype=bool)
    tile_last = np.zeros(NP, dtype=bool)
    tile_first[0] = True
    for i in range(1, NP):
        if e_tl[i] != e_tl[i - 1]:
            tile_first[i] = True
            tile_last[i - 1] = True
    tile_last[NP - 1] = True

    groups = []
    for g in range(ngroups):
        g0 = int(g_bounds[g])
        g1 = int(g_bounds[g + 1])
        sections = []
        for j in range(NCHUNKS):
            s = g * NCHUNKS + j
            sections.append((int(blk_base[s]), int(nblk[s])))
        in_g = (e_tl >= g0) & (e_tl < g1)
        mbs = np.nonzero(in_g)[0]
        ms0, ms1 = int(mbs.min()), int(mbs.max()) + 1
        tile_pieces = []
        for t in range(g0, g1):
            sel = np.nonzero(e_tl == t)[0]
            tile_pieces.append(
                [
                    (int(mb), int(e_babs[mb]), int(e_j[mb]),
                     bool(tile_first[mb]), bool(tile_last[mb]))
                    for mb in sel
                ]
            )
        groups.append((g0, g1, sections, ms0, ms1, tile_pieces))

    # per-core payloads
    per_core = []
    for c in range(n_cores):
        slot, babs, tc, dc, vc, colc, pk = per_core_raw[c]
        idx_flat = np.full(TOT, cc, dtype=np.int16)  # pad -> zero row
        idx_flat[slot] = colc
        idx16 = np.tile(np.ascontiguousarray(idx_flat.reshape(-1, 16).T), (8, 1))
        # edge -> emission mb
        pos = np.searchsorted(pk_u, pk)
        mb_e = mb_of_rank[pos]
        S8 = np.zeros((P, NP, P), dtype=ml_dtypes.float8_e3m4)
        S8[slot % P, mb_e, dc] = vc.astype(np.float16)
        per_core.append(
            {
                "idx16": np.ascontiguousarray(idx16),
                "S8": np.ascontiguousarray(S8.reshape(P, NP * P)),
            }
        )

    sched = {
        "tiles": tiles,
        "npc": npc,
        "cc": cc,
        "groups": groups,
        "NBg": NBg,
        "TOT": TOT,
        "NP": NP,
    }
    return sched, per_core


def _raw_gather_128(eng, mybir, out_ap, in_ap, idxs_ap, num_idxs, queue_num,
                    num_idxs_reg=None):
    """dma_gather with a 128-byte element on a 256-byte-stride table.

    Mirrors bass's dma_gather (non-transpose, DRAM source, no prepare)
    but skips its 256B-element assert: the SWDGE ucode packetizes any
    elem_size (packet = min(elem_size_bytes, 16K)); only the row stride
    must be a 256B multiple (stride_bytes_256 field).
    """
    eng._assert_queue_num(queue_num)
    elem_size = 128  # fp8 elements = 128 bytes
    elem_step = 256  # table row stride in fp8 elements = 256 bytes
    assert in_ap.ap[0][0] == elem_step, in_ap.ap
    assert in_ap.ap[-1][1] == elem_size, in_ap.ap
    assert out_ap.ap[-1][1] == elem_size, out_ap.ap
    assert out_ap.ap[0][1] * out_ap.ap[1][1] == num_idxs, out_ap.ap
    _in_ap = eng.lower_ap_dma(in_ap, for_custom_bir_dma=True)
    _idxs_ap = eng.lower_ap(idxs_ap)
    _out_ap = eng.lower_ap(out_ap)
    return eng.add_instruction(
        mybir.InstDMAGatherAnt(
            name=eng.bass.get_next_instruction_name(),
            ins=[
                *_in_ap,
                _idxs_ap,
                eng.lower_val_access(
                    eng.to_reg(
                        num_idxs if num_idxs_reg is None else num_idxs_reg
                    )
                ),
            ],
            outs=[_out_ap],
            transpose=False,
            num_idxs=num_idxs,
            elem_size=elem_size,
            stride_bytes_256=1,
            gen_mode=0,
            single_packet=False,
            queue_num=queue_num,
            sbuf_tokens_per_rank=0,
            sbuf_free_dim_per_rank=0,
            sbuf_free_dim_pad_per_rank=0,
            sbuf_byte_offset=0,
        )
    )


def _build_program(n_nodes, sched):
    import concourse.bacc as bacc
    import concourse.mybir as mybir
    import concourse.tile as tile

    f32 = mybir.dt.float32
    f16 = mybir.dt.float16
    f8 = mybir.dt.float8e3
    i16 = mybir.dt.int16

    npc = sched["npc"]
    cc = sched["cc"]
    TOT = sched["TOT"]
    NP = sched["NP"]

    nc = bacc.Bacc(num_swdge_queues=4)
    feat8 = [
        nc.dram_tensor(f"feat8_{j}", [cc + 1, 2, P], f8, kind="ExternalInput")
        for j in range(NCHUNKS)
    ]
    featT = nc.dram_tensor("featT", [D, npc], f16, kind="ExternalInput")
    w1 = nc.dram_tensor("W1", [D, D], f16, kind="ExternalInput")
    w2 = nc.dram_tensor("W2", [D, D], f16, kind="ExternalInput")
    bsum = nc.dram_tensor("bsum", [D, 2], f32, kind="ExternalInput")
    idx16 = nc.dram_tensor("idx16", [P, TOT // 16], i16, kind="ExternalInput")
    s8d = nc.dram_tensor("S8", [P, NP * P], f8, kind="ExternalInput")
    outT = nc.dram_tensor("outT", [D, npc], f16, kind="ExternalOutput")

    with tile.TileContext(nc) as tc:
        with (
            tc.tile_pool(name="const", bufs=1) as constp,
            tc.tile_pool(name="gpool", bufs=8) as gpool,
            tc.tile_pool(name="spool", bufs=3) as spool,
            tc.tile_pool(name="ftpool", bufs=4) as ftpool,
            tc.tile_pool(name="xspool", bufs=4) as xspool,
            tc.tile_pool(name="ampool", bufs=6) as ampool,
            tc.tile_pool(name="ostage", bufs=3) as ostagep,
            tc.tile_pool(name="psx", bufs=6, space="PSUM") as psx,
            tc.tile_pool(name="pso", bufs=2, space="PSUM") as pso,
        ):
            # --- constants.  idx16 is loaded in per-group slices, two
            # groups ahead of use, so no gather ever waits on a bulk
            # index transfer (an 11.9us pipeline bubble otherwise).
            grp_rng = [
                (g[2][0][0], g[2][-1][0] + g[2][-1][1]) for g in sched["groups"]
            ]
            idx16_t = constp.tile([P, TOT // 16], i16)

            def load_idx_slice(g):
                b0, b1 = grp_rng[g]
                nc.scalar.dma_start(
                    out=idx16_t[:, b0 * 8 : b1 * 8], in_=idx16[:, b0 * 8 : b1 * 8]
                )

            load_idx_slice(0)
            load_idx_slice(1)
            w1_t = constp.tile([P, P], f16)
            nc.sync.dma_start(out=w1_t[:], in_=w1[:, :])
            w2_t = constp.tile([P, P], f16)
            nc.sync.dma_start(out=w2_t[:], in_=w2[:, :])
            bias_t = constp.tile([P, 2], f32)
            nc.sync.dma_start(out=bias_t[:], in_=bsum[:, :])

            def emit_epilogue(g0, gw, fT, xS, oT):
                aT = ampool.tile([P, gw], f16, tag="aT")
                mT = ampool.tile([P, gw], f16, tag="mT")
                nc.vector.tensor_tensor(
                    out=aT[:], in0=xS[:, :gw], in1=fT[:, :gw],
                    op=mybir.AluOpType.add,
                )
                nc.vector.tensor_tensor(
                    out=mT[:], in0=xS[:, :gw], in1=fT[:, :gw],
                    op=mybir.AluOpType.mult,
                )
                for c0 in range(0, gw, 512):
                    cw = min(512, gw - c0)
                    out2 = pso.tile([P, 512], f32, tag="out2")
                    nc.tensor.matmul(
                        out=out2[:, :cw], lhsT=w1_t[:], rhs=aT[:, c0 : c0 + cw],
                        start=True, stop=False,
                    )
                    nc.tensor.matmul(
                        out=out2[:, :cw], lhsT=w2_t[:], rhs=mT[:, c0 : c0 + cw],
                        start=False, stop=True,
                    )
                    nc.scalar.activation(
                        out=oT[:, c0 : c0 + cw],
                        in_=out2[:, :cw],
                        func=mybir.ActivationFunctionType.Identity,
                        bias=bias_t[:, 0:1],
                        scale=1.0,
                    )
                nc.scalar.dma_start(
                    out=outT[:, g0 * P : g0 * P + gw], in_=oT[:, :gw]
                )

            prev = None
            for gi, (g0, g1, sections, ms0, ms1, tile_pieces) in enumerate(
                sched["groups"]
            ):
                gw = min(g1 * P, npc) - g0 * P
                npg = ms1 - ms0
                if gi + 2 < len(sched["groups"]):
                    load_idx_slice(gi + 2)

                # one dma_gather per feature-table chunk, parallel SWDGE
                # queues; 128B fp8 payload per edge from 256B-stride rows
                gtiles = {}
                for j in range(NCHUNKS):
                    sec_start, sec_nblk = sections[j]
                    G = gpool.tile([P, sec_nblk, P], f8, tag=f"G{j}")
                    n_idx = sec_nblk * P
                    _raw_gather_128(
                        nc.gpsimd,
                        mybir,
                        G[:],
                        feat8[j][:, 0, :],
                        idx16_t[:, sec_start * 8 : sec_start * 8 + n_idx // 16],
                        n_idx,
                        queue_num=j,
                    )
                    gtiles[j] = (G, sec_start)

                # host-built one-hot*val S blocks, streamed on SP ring
                S = spool.tile([P, npg, P], f8, tag="S")
                nc.sync.dma_start(out=S[:], in_=s8d[:, ms0 * P : ms1 * P])

                # featT slice for this group, on the ACT HWDGE ring
                fT = ftpool.tile([P, gw], f16, tag="fT")
                nc.scalar.dma_start(out=fT[:], in_=featT[:, g0 * P : g0 * P + gw])

                # dense epilogue for the PREVIOUS group, emitted BEFORE this
                # group's segment matmuls: its inputs (xS/fT of g-1) are
                # ready, so the PE spends the wait for this group's gather
                # drain on epilogue work instead of idling, and the
                # post-final-gather tail shrinks by one group's epilogue.
                # (The old after-matmuls ordering existed for the on-chip
                # DVE S-build, which is gone.)
                if prev is not None:
                    emit_epilogue(*prev)

                # xT psum per tile; evict to fp16 SBUF on the scalar engine
                xS = xspool.tile([P, gw], f16, tag="xS")
                for t in range(g0, g1):
                    w = min((t + 1) * P, npc) - t * P
                    pieces = tile_pieces[t - g0]
                    xT = psx.tile([P, P], f32, tag="xT")
                    for mb, babs, j, st, sp in pieces:
                        G, sec_start = gtiles[j]
                        nc.tensor.matmul(
                            out=xT[:],
                            lhsT=G[:, babs - sec_start, :],
                            rhs=S[:, mb - ms0, :],
                            start=st,
                            stop=sp,
                        )
                    toff = (t - g0) * P
                    nc.scalar.activation(
                        out=xS[:, toff : toff + w],
                        in_=xT[:, :w],
                        func=mybir.ActivationFunctionType.Identity,
                        bias=bias_t[:, 1:2],
                        scale=1.0,
                    )

                oT = ostagep.tile([P, gw], f16, tag="oT")
                prev = (g0, gw, fT, xS, oT)

            emit_epilogue(*prev)
    nc.compile()
    return nc


def _run(rows, cols, vals, features, W1, b1, W2, b2, n_nodes, n_cores):
    global _LAST_RESULTS
    import ml_dtypes
    from concourse import bass_utils

    npc = n_nodes // n_cores
    features = np.ascontiguousarray(np.asarray(features, dtype=np.float32))
    W1_16 = np.ascontiguousarray(np.asarray(W1, dtype=np.float32).astype(np.float16))
    W2_16 = np.ascontiguousarray(np.asarray(W2, dtype=np.float32).astype(np.float16))
    bsum = np.zeros((D, 2), dtype=np.float32)
    bsum[:, 0] = np.asarray(b1, dtype=np.float32) + np.asarray(b2, dtype=np.float32)
    bsum = np.ascontiguousarray(bsum)

    sched, per_core = _prep(rows, cols, vals, n_nodes, n_cores)
    nc = _build_program(n_nodes, sched)

    cc = sched["cc"]
    feat8_flat = features.astype(np.float16).astype(ml_dtypes.float8_e3m4)
    feat8_chunks = []
    for j in range(NCHUNKS):
        chunk = np.zeros((cc + 1, 2, P), dtype=ml_dtypes.float8_e3m4)
        chunk[:cc, 0, :] = feat8_flat[j * cc : (j + 1) * cc, :]
        feat8_chunks.append(np.ascontiguousarray(chunk))

    in_maps = []
    for c in range(n_cores):
        featT_c = np.ascontiguousarray(
            features[c * npc : (c + 1) * npc, :].T.astype(np.float16)
        )
        im = {
            "featT": featT_c,
            "W1": W1_16,
            "W2": W2_16,
            "bsum": bsum,
            "idx16": per_core[c]["idx16"],
            "S8": per_core[c]["S8"],
        }
        for j in range(NCHUNKS):
            im[f"feat8_{j}"] = feat8_chunks[j]
        in_maps.append(im)

    res = bass_utils.run_bass_kernel_spmd(nc, in_maps, core_ids=list(range(n_cores)))
    _LAST_RESULTS = res
    out = np.concatenate(
        [r["outT"].T.astype(np.float32) for r in res.results], axis=0
    )
    return np.ascontiguousarray(out)


def kernel(rows, cols, vals, features, W1, b1, W2, b2):
    return _run(rows, cols, vals, features, W1, b1, W2, b2, N_NODES, N_CORES)
